# revision 14
# baseline (speedup 1.0000x reference)
"""Masked dot-product attention (B=64, Lq=Lk=1024, d=64, fp32) on 8 TRN2 cores.

v2 strategy (per core: 8 batch slots, ragged k-tiles, sorted+dealt):
  - All inputs bf16. Host folds 1/sqrt(d) into Q. Masking is NOT in the
    score matmul: dead k rows (k >= valid_len) are zeroed in V (including
    the ones-column that produces softmax denominators), so whatever the
    exp stage emits for dead scores is multiplied by zero in the O matmul.
  - S^T[k,q] per k-tile via bf16 matmul (contraction d=64), PSUM f32.
  - exp is split across TWO engines to break the single-engine exp wall:
      ACT: exact exp (PSUM->SBUF bf16)
      DVE: Schraudolph fast-exp: i16 = rint(S*(2^7/ln2) + 127*2^7), whose
           bit pattern IS bf16(exp(S)) to ~3%; f32->i16 convert saturates
           (verified on HW) so dead scores (~-1e6) become 0x8000 = -0.0.
           The +3%-band bias cancels in the softmax division; using the
           uncorrected constant keeps exp(0)=1.0 exactly so valid_len==0
           batches (host zeroes Q) stay exactly uniform.
    Small batches (<=2 k-tiles) are ACT-only: Schraudolph error hurts most
    when few keys are live.
  - O^T[q,j] = sum_k P^T[k,q-chunk]^T V[k,j]: lhsT = P^T chunk [128,128],
    rhs = V-tile [128,65] (64 dims + ones column) -> out [128q, 65], only
    65 PE rows per matmul (vs 1024 streaming V^T P). PSUM accumulation
    groups clear has_written bank-wide on start, so the 8 q-chunks run as
    2 passes x 4 chunks, each chunk in its own PSUM bank ([128,4,512] f32
    tile, single buffer); pass1 re-reads the kept P tiles. O-work is a
    global FIFO drained between tiles so the PE stream never blocks on a
    PSUM buffer freed by later instructions.
  - copies PSUM->SBUF (engine chosen by load balance), output DMAs issued
    from GpSimd (SWDGE) keeping SP.SEQ/HWDGE for inputs only.
"""

import math
from collections import deque

import numpy as np
import ml_dtypes

import concourse.bass as bass
import concourse.mybir as mybir
import concourse.tile as tile
from concourse import bacc
from concourse.bass_utils import run_bass_kernel_spmd

N_CORES = 8
B = 64
L = 1024
D = 64
BPC = B // N_CORES
KT = L // 128

F32 = mybir.dt.float32
BF16 = mybir.dt.bfloat16
I16 = mybir.dt.int16
BF16NP = ml_dtypes.bfloat16

A16 = 128.0 / math.log(2.0)   # 184.6617
B16 = 127.0 * 128.0           # 16256.0

ACT_EXP_NS = 1038.0
DVE_EXP_NS = 1191.0
ACT_CP_NS = 402.0
DVE_CP_NS = 396.0

_prog_cache = {}


def _plan(ns):
    """Execution order, per-tile engine map, per-copy engine map."""
    # head: a small ACT-only batch; tail: the smallest batch (short drain).
    exec_order = ([BPC - 2] + list(range(BPC - 2)) + [BPC - 1])
    # zipper the first two batches so both exp engines start immediately
    b0, b1 = exec_order[0], exec_order[1]
    z = []
    t0 = [(b0, kt) for kt in range(ns[b0])]
    t1 = [(b1, kt) for kt in range(ns[b1])]
    while t0 or t1:
        if t0:
            z.append(t0.pop(0))
        if t1:
            z.append(t1.pop(0))
    tiles = z + [(b, kt) for b in exec_order[2:] for kt in range(ns[b])]
    busy = {"A": 0.0, "D": 0.0}
    eng = {}
    for (b, kt) in tiles:
        if ns[b] <= 2:
            e = "A"   # accuracy: few live keys -> exact exp
        elif b == b1 and kt < 2:
            e = "D"   # wake DVE early in the head
        elif busy["A"] + ACT_EXP_NS <= busy["D"] + DVE_EXP_NS:
            e = "A"
        else:
            e = "D"
        eng[(b, kt)] = e
        busy[e] += ACT_EXP_NS if e == "A" else DVE_EXP_NS
    cpeng = {}
    for b in exec_order:
        for p in range(2):
            if busy["A"] + ACT_CP_NS <= busy["D"] + DVE_CP_NS:
                cpeng[(b, p)] = "A"
                busy["A"] += ACT_CP_NS
            else:
                cpeng[(b, p)] = "D"
                busy["D"] += DVE_CP_NS
    return exec_order, tiles, eng, cpeng


def _build_program(ns):
    """ns: per-slot k-tile counts (tuple of BPC ints in 1..KT)."""
    nc = bacc.Bacc("TRN2", target_bir_lowering=False, debug=False,
                   num_devices=N_CORES)
    exec_order, tiles, eng, cpeng = _plan(ns)

    # qkt: [ktile0 (128) | qt (1024) | ktile1.. (896)] bf16, 64 partitions
    qkt_d = nc.dram_tensor("qkt", [BPC, D, 2 * L + 128], BF16,
                           kind="ExternalInput")
    vp_d = nc.dram_tensor("vp", [BPC, 128, KT, D + 1], BF16,
                          kind="ExternalInput")
    o_d = nc.dram_tensor("o", [BPC, 128, 2, 4, D + 1], F32,
                         kind="ExternalOutput")

    with tile.TileContext(nc) as tc:
        with (
            tc.tile_pool(name="qk", bufs=1) as qk_pool,
            tc.tile_pool(name="vpp", bufs=1) as vp_pool,
            tc.tile_pool(name="pt", bufs=12) as pt_pool,
            tc.tile_pool(name="osb", bufs=3) as osb_pool,
            tc.tile_pool(name="sp", bufs=3, space="PSUM") as sp_pool,
            tc.tile_pool(name="op", bufs=1, space="PSUM") as op_pool,
        ):
            qkt_s = {}
            vp_s = {}
            head2 = set(exec_order[:2])
            for b in exec_order:
                nkt = ns[b]
                q_t = qk_pool.tile([D, 2 * L + 128], BF16, tag=f"qkt{b}")
                v_t = vp_pool.tile([128, KT, D + 1], BF16, tag=f"vp{b}")
                qkt_s[b] = q_t
                vp_s[b] = v_t
            # head: first two batches get small first chunks so their first
            # S-matmuls (and both exp engines) start as early as possible
            for b in exec_order[:2]:
                nc.sync.dma_start(qkt_s[b][:, :384], qkt_d[b][:, :384])
            for b in exec_order[:2]:
                nkt = ns[b]
                end = 128 + L + (nkt - 1) * 128
                nc.sync.dma_start(vp_s[b][:, :nkt, :], vp_d[b][:, :nkt, :])
                nc.sync.dma_start(qkt_s[b][:, 384:end], qkt_d[b][:, 384:end])
            for b in exec_order[2:]:
                nkt = ns[b]
                end = 128 + L + (nkt - 1) * 128
                nc.sync.dma_start(qkt_s[b][:, :end], qkt_d[b][:, :end])
                nc.sync.dma_start(vp_s[b][:, :nkt, :], vp_d[b][:, :nkt, :])

            def ktm_sl(b, kt):
                if kt == 0:
                    return qkt_s[b][:, :128]
                o = 128 + L + (kt - 1) * 128
                return qkt_s[b][:, o:o + 128]

            def qt_sl(b):
                return qkt_s[b][:, 128:128 + L]

            state = {}  # b -> dict(op0/op1 tiles, osb, pt list)
            owork = deque()

            def emit_exp(b, kt, sp, pt, splits):
                e = eng[(b, kt)]
                if e == "A":
                    for (lo, hi) in splits:
                        nc.scalar.activation(
                            pt[:, lo:hi].bitcast(BF16), sp[:, lo:hi],
                            mybir.ActivationFunctionType.Exp)
                else:
                    for (lo, hi) in splits:
                        nc.vector.tensor_scalar(
                            pt[:, lo:hi], sp[:, lo:hi], A16, B16,
                            mybir.AluOpType.mult, mybir.AluOpType.add)

            last_b = exec_order[-1]

            def o_unit_chunk(b, cc):
                # Deferred O accumulation for q-chunk cc. Chunks sharing a
                # PSUM bank run as SEQUENTIAL groups (all of chunk c's
                # matmuls before chunk c+1 starts), which is legal despite
                # the bank-wide has_written clear on each group start.
                def f():
                    st = state[b]
                    nkt = ns[b]
                    bk, c = divmod(cc, 4)
                    if cc == 0:
                        st["op"] = op_pool.tile([128, 2, 4, 128], F32,
                                                tag="op", name=f"op_{b}")
                    op_t = st["op"]
                    for kt in range(nkt):
                        nc.tensor.matmul(
                            op_t[:, bk, c, :D + 1],
                            st["pt"][kt][:, cc * 128:(cc + 1) * 128]
                            .bitcast(BF16),
                            vp_s[b][:, kt, :],
                            start=(kt == 0), stop=(kt == nkt - 1))
                    if c == 3:
                        e = cpeng[(b, bk)]
                        dst = st["osb"][:, bk, :, :]
                        src = op_t[:, bk, :, :D + 1]
                        if e == "A":
                            nc.scalar.copy(dst, src)
                        else:
                            nc.vector.tensor_copy(dst, src)
                        dma_eng = nc.sync if b == last_b else nc.gpsimd
                        dma_eng.dma_start(o_d[b][:, bk, :, :], dst)
                return f

            for i, (b, kt) in enumerate(tiles):
                nkt = ns[b]
                if kt == 0:
                    state[b] = {
                        "pt": [],
                        "osb": osb_pool.tile([128, 2, 4, D + 1], F32,
                                             tag="osb", name=f"osb{b}"),
                    }
                sp = sp_pool.tile([128, L], F32, tag="sp")
                if i < 2:
                    jobs = [(0, 256), (256, 512), (512, 1024)]
                else:
                    jobs = [(0, 512), (512, 1024)]
                for (lo, hi) in jobs:
                    nc.tensor.matmul(sp[:, lo:hi], ktm_sl(b, kt),
                                     qt_sl(b)[:, lo:hi],
                                     start=True, stop=True)
                pt = pt_pool.tile([128, L], I16, tag="pt")
                state[b]["pt"].append(pt)
                emit_exp(b, kt, sp, pt, jobs)
                if kt == nkt - 1:
                    # O work for batch b becomes eligible a few tiles
                    # later: by then its last exp has completed (sp
                    # recycling bounds exp lag), so the in-order PE queue
                    # never stalls on it; chunks stagger to avoid bursts.
                    for j in range(8):
                        owork.append((i + 3 + j // 3, o_unit_chunk(b, j)))
                while owork and owork[0][0] <= i:
                    owork.popleft()[1]()
            while owork:
                owork.popleft()[1]()

    nc.compile()
    return nc


def get_program(ns):
    ns = tuple(ns)
    if ns not in _prog_cache:
        _prog_cache[ns] = _build_program(ns)
    return _prog_cache[ns]


def _prep_inputs(q, k, v, vl):
    """q,k,v: [n, L, D] f32; vl: [n] int. Returns (qkt, vp) bf16 arrays."""
    n = q.shape[0]
    qkt = np.zeros((n, D, 2 * L + 128), BF16NP)
    qt = (q.transpose(0, 2, 1) * np.float32(1.0 / np.sqrt(D))).astype(BF16NP)
    zmask = vl == 0
    if zmask.any():
        qt[zmask] = 0
    ktm = k.transpose(0, 2, 1).astype(BF16NP)
    qkt[:, :, :128] = ktm[:, :, :128]
    qkt[:, :, 128:128 + L] = qt
    qkt[:, :, 128 + L:2 * L] = ktm[:, :, 128:]
    vp = np.empty((n, L, D + 1), np.float32)
    vp[:, :, :D] = v
    vp[:, :, D] = 1.0
    iota = np.arange(L)
    dead = (iota[None, :] >= vl[:, None]) & ~zmask[:, None]
    vp[dead] = 0.0
    vp = vp.astype(BF16NP)
    vp = np.ascontiguousarray(
        vp.reshape(n, KT, 128, D + 1).transpose(0, 2, 1, 3))
    return qkt, vp


def kernel(queries, keys, values, valid_lens):
    queries = np.asarray(queries, np.float32)
    keys = np.asarray(keys, np.float32)
    values = np.asarray(values, np.float32)
    vl = np.asarray(valid_lens).astype(np.int64)

    # Ragged load balancing: sort batches by active k-tile count descending,
    # deal across cores; slot s runs max-of-group tiles on every core.
    nact = np.where(vl == 0, KT, -(-vl // 128)).astype(np.int64)
    order = np.argsort(-nact, kind="stable")
    ns = tuple(int(nact[order[s * N_CORES]]) for s in range(BPC))

    qkt, vp = _prep_inputs(queries[order], keys[order], values[order],
                           vl[order])

    nc = get_program(ns)
    in_maps = []
    for c in range(N_CORES):
        idx = [s * N_CORES + c for s in range(BPC)]
        in_maps.append({
            "qkt": np.ascontiguousarray(qkt[idx]),
            "vp": np.ascontiguousarray(vp[idx]),
        })

    res = None
    for attempt in range(3):
        try:
            res = run_bass_kernel_spmd(nc, in_maps, list(range(N_CORES)))
            break
        except Exception:
            if attempt == 2:
                raise
            import time as _time
            _time.sleep(2.0)
            try:
                import jax
                jax.clear_caches()
            except Exception:
                pass

    out = np.empty((B, L, D), np.float32)
    for c in range(N_CORES):
        o = res.results[c]["o"]  # [BPC, 128, 2, 4, D+1]
        o = np.asarray(o, np.float32).reshape(BPC, 128, KT, D + 1)
        o = o.transpose(0, 2, 1, 3).reshape(BPC, L, D + 1)
        on = o[:, :, :D] / o[:, :, D:D + 1]
        for s in range(BPC):
            out[order[s * N_CORES + c]] = on[s]
    return out


# revision 15
# speedup vs baseline: 1.0053x; 1.0053x over previous
"""Masked dot-product attention (B=64, Lq=Lk=1024, d=64, fp32) on 8 TRN2 cores.

v2 strategy (per core: 8 batch slots, ragged k-tiles, sorted+dealt):
  - All inputs bf16. Host folds 1/sqrt(d) into Q. Masking is NOT in the
    score matmul: dead k rows (k >= valid_len) are zeroed in V (including
    the ones-column that produces softmax denominators), so whatever the
    exp stage emits for dead scores is multiplied by zero in the O matmul.
  - S^T[k,q] per k-tile via bf16 matmul (contraction d=64), PSUM f32.
  - exp is split across TWO engines to break the single-engine exp wall:
      ACT: exact exp (PSUM->SBUF bf16)
      DVE: Schraudolph fast-exp: i16 = rint(S*(2^7/ln2) + 127*2^7), whose
           bit pattern IS bf16(exp(S)) to ~3%; f32->i16 convert saturates
           (verified on HW) so dead scores (~-1e6) become 0x8000 = -0.0.
           The +3%-band bias cancels in the softmax division; using the
           uncorrected constant keeps exp(0)=1.0 exactly so valid_len==0
           batches (host zeroes Q) stay exactly uniform.
    Small batches (<=2 k-tiles) are ACT-only: Schraudolph error hurts most
    when few keys are live.
  - O^T[q,j] = sum_k P^T[k,q-chunk]^T V[k,j]: lhsT = P^T chunk [128,128],
    rhs = V-tile [128,65] (64 dims + ones column) -> out [128q, 65], only
    65 PE rows per matmul (vs 1024 streaming V^T P). PSUM accumulation
    groups clear has_written bank-wide on start, so the 8 q-chunks run as
    2 passes x 4 chunks, each chunk in its own PSUM bank ([128,4,512] f32
    tile, single buffer); pass1 re-reads the kept P tiles. O-work is a
    global FIFO drained between tiles so the PE stream never blocks on a
    PSUM buffer freed by later instructions.
  - copies PSUM->SBUF (engine chosen by load balance), output DMAs issued
    from GpSimd (SWDGE) keeping SP.SEQ/HWDGE for inputs only.
"""

import math
from collections import deque

import numpy as np
import ml_dtypes

import concourse.bass as bass
import concourse.mybir as mybir
import concourse.tile as tile
from concourse import bacc
from concourse.bass_utils import run_bass_kernel_spmd

N_CORES = 8
B = 64
L = 1024
D = 64
BPC = B // N_CORES
KT = L // 128

F32 = mybir.dt.float32
BF16 = mybir.dt.bfloat16
I16 = mybir.dt.int16
BF16NP = ml_dtypes.bfloat16

A16 = 128.0 / math.log(2.0)   # 184.6617
B16 = 127.0 * 128.0           # 16256.0

ACT_EXP_NS = 1038.0
DVE_EXP_NS = 1191.0
ACT_CP_NS = 402.0
DVE_CP_NS = 396.0

_prog_cache = {}


def _plan(ns):
    """Execution order, per-tile engine map, per-copy engine map."""
    # head: a small ACT-only batch; tail: the smallest batch (short drain).
    exec_order = ([BPC - 2] + list(range(BPC - 2)) + [BPC - 1])
    # zipper the first two batches so both exp engines start immediately
    b0, b1 = exec_order[0], exec_order[1]
    z = []
    t0 = [(b0, kt) for kt in range(ns[b0])]
    t1 = [(b1, kt) for kt in range(ns[b1])]
    while t0 or t1:
        if t0:
            z.append(t0.pop(0))
        if t1:
            z.append(t1.pop(0))
    tiles = z + [(b, kt) for b in exec_order[2:] for kt in range(ns[b])]
    busy = {"A": 0.0, "D": 0.0}
    eng = {}
    for (b, kt) in tiles:
        if ns[b] <= 2:
            e = "A"   # accuracy: few live keys -> exact exp
        elif b == b1 and kt < 2:
            e = "D"   # wake DVE early in the head
        elif busy["A"] + ACT_EXP_NS <= busy["D"] + DVE_EXP_NS:
            e = "A"
        else:
            e = "D"
        eng[(b, kt)] = e
        busy[e] += ACT_EXP_NS if e == "A" else DVE_EXP_NS
    cpeng = {}
    for b in exec_order:
        for p in range(2):
            if busy["A"] + ACT_CP_NS <= busy["D"] + DVE_CP_NS:
                cpeng[(b, p)] = "A"
                busy["A"] += ACT_CP_NS
            else:
                cpeng[(b, p)] = "D"
                busy["D"] += DVE_CP_NS
    return exec_order, tiles, eng, cpeng


def _build_program(ns):
    """ns: per-slot k-tile counts (tuple of BPC ints in 1..KT)."""
    nc = bacc.Bacc("TRN2", target_bir_lowering=False, debug=False,
                   num_devices=N_CORES)
    exec_order, tiles, eng, cpeng = _plan(ns)

    # qkt: [ktile0 (128) | qt (1024) | ktile1.. (896)] bf16, 64 partitions
    qkt_d = nc.dram_tensor("qkt", [BPC, D, 2 * L + 128], BF16,
                           kind="ExternalInput")
    vp_d = nc.dram_tensor("vp", [BPC, 128, KT, D + 1], BF16,
                          kind="ExternalInput")
    o_d = nc.dram_tensor("o", [BPC, 128, 2, 4, D + 1], F32,
                         kind="ExternalOutput")

    with tile.TileContext(nc) as tc:
        with (
            tc.tile_pool(name="qk", bufs=1) as qk_pool,
            tc.tile_pool(name="vpp", bufs=1) as vp_pool,
            tc.tile_pool(name="pt", bufs=12) as pt_pool,
            tc.tile_pool(name="osb", bufs=3) as osb_pool,
            tc.tile_pool(name="sp", bufs=3, space="PSUM") as sp_pool,
            tc.tile_pool(name="op", bufs=1, space="PSUM") as op_pool,
        ):
            qkt_s = {}
            vp_s = {}
            head2 = set(exec_order[:2])
            for b in exec_order:
                nkt = ns[b]
                q_t = qk_pool.tile([D, 2 * L + 128], BF16, tag=f"qkt{b}")
                v_t = vp_pool.tile([128, KT, D + 1], BF16, tag=f"vp{b}")
                qkt_s[b] = q_t
                vp_s[b] = v_t
            # head: first two batches get small first chunks so their first
            # S-matmuls (and both exp engines) start as early as possible
            for b in exec_order[:2]:
                nc.sync.dma_start(qkt_s[b][:, :384], qkt_d[b][:, :384])
            for b in exec_order[:2]:
                nkt = ns[b]
                end = 128 + L + (nkt - 1) * 128
                nc.sync.dma_start(vp_s[b][:, :nkt, :], vp_d[b][:, :nkt, :])
                nc.sync.dma_start(qkt_s[b][:, 384:end], qkt_d[b][:, 384:end])
            for b in exec_order[2:]:
                nkt = ns[b]
                end = 128 + L + (nkt - 1) * 128
                nc.sync.dma_start(qkt_s[b][:, :end], qkt_d[b][:, :end])
                nc.sync.dma_start(vp_s[b][:, :nkt, :], vp_d[b][:, :nkt, :])

            def ktm_sl(b, kt):
                if kt == 0:
                    return qkt_s[b][:, :128]
                o = 128 + L + (kt - 1) * 128
                return qkt_s[b][:, o:o + 128]

            def qt_sl(b):
                return qkt_s[b][:, 128:128 + L]

            state = {}  # b -> dict(op0/op1 tiles, osb, pt list)
            owork = deque()

            def emit_exp(b, kt, sp, pt, splits):
                e = eng[(b, kt)]
                if e == "A":
                    for (lo, hi) in splits:
                        nc.scalar.activation(
                            pt[:, lo:hi].bitcast(BF16), sp[:, lo:hi],
                            mybir.ActivationFunctionType.Exp)
                else:
                    for (lo, hi) in splits:
                        nc.vector.tensor_scalar(
                            pt[:, lo:hi], sp[:, lo:hi], A16, B16,
                            mybir.AluOpType.mult, mybir.AluOpType.add)

            last_b = exec_order[-1]

            def o_unit_chunk(b, cc):
                # Deferred O accumulation for q-chunk cc. Chunks sharing a
                # PSUM bank run as SEQUENTIAL groups (all of chunk c's
                # matmuls before chunk c+1 starts), which is legal despite
                # the bank-wide has_written clear on each group start.
                def f():
                    st = state[b]
                    nkt = ns[b]
                    bk, c = divmod(cc, 4)
                    if cc == 0:
                        st["op"] = op_pool.tile([128, 2, 4, 128], F32,
                                                tag="op", name=f"op_{b}")
                    op_t = st["op"]
                    for kt in range(nkt):
                        nc.tensor.matmul(
                            op_t[:, bk, c, :D + 1],
                            st["pt"][kt][:, cc * 128:(cc + 1) * 128]
                            .bitcast(BF16),
                            vp_s[b][:, kt, :],
                            start=(kt == 0), stop=(kt == nkt - 1))
                    if c == 3:
                        e = cpeng[(b, bk)]
                        dst = st["osb"][:, bk, :, :]
                        src = op_t[:, bk, :, :D + 1]
                        if e == "A":
                            nc.scalar.copy(dst, src)
                        else:
                            nc.vector.tensor_copy(dst, src)
                        dma_eng = nc.sync if b == last_b else nc.gpsimd
                        dma_eng.dma_start(o_d[b][:, bk, :, :], dst)
                return f

            for i, (b, kt) in enumerate(tiles):
                nkt = ns[b]
                if kt == 0:
                    state[b] = {
                        "pt": [],
                        "osb": osb_pool.tile([128, 2, 4, D + 1], F32,
                                             tag="osb", name=f"osb{b}"),
                    }
                sp = sp_pool.tile([128, L], F32, tag="sp")
                if i < 2:
                    jobs = [(0, 256), (256, 512), (512, 1024)]
                    esplits = jobs
                else:
                    jobs = [(0, 512), (512, 1024)]
                    esplits = [(0, 1024)]
                for (lo, hi) in jobs:
                    nc.tensor.matmul(sp[:, lo:hi], ktm_sl(b, kt),
                                     qt_sl(b)[:, lo:hi],
                                     start=True, stop=True)
                pt = pt_pool.tile([128, L], I16, tag="pt")
                state[b]["pt"].append(pt)
                emit_exp(b, kt, sp, pt, esplits)
                if kt == nkt - 1:
                    # O work for batch b becomes eligible a few tiles
                    # later: by then its last exp has completed (sp
                    # recycling bounds exp lag), so the in-order PE queue
                    # never stalls on it; chunks stagger to avoid bursts.
                    for j in range(8):
                        owork.append((i + 3 + j // 3, o_unit_chunk(b, j)))
                while owork and owork[0][0] <= i:
                    owork.popleft()[1]()
            while owork:
                owork.popleft()[1]()

    nc.compile()
    return nc


def get_program(ns):
    ns = tuple(ns)
    if ns not in _prog_cache:
        _prog_cache[ns] = _build_program(ns)
    return _prog_cache[ns]


def _prep_inputs(q, k, v, vl):
    """q,k,v: [n, L, D] f32; vl: [n] int. Returns (qkt, vp) bf16 arrays."""
    n = q.shape[0]
    qkt = np.zeros((n, D, 2 * L + 128), BF16NP)
    qt = (q.transpose(0, 2, 1) * np.float32(1.0 / np.sqrt(D))).astype(BF16NP)
    zmask = vl == 0
    if zmask.any():
        qt[zmask] = 0
    ktm = k.transpose(0, 2, 1).astype(BF16NP)
    qkt[:, :, :128] = ktm[:, :, :128]
    qkt[:, :, 128:128 + L] = qt
    qkt[:, :, 128 + L:2 * L] = ktm[:, :, 128:]
    vp = np.empty((n, L, D + 1), np.float32)
    vp[:, :, :D] = v
    vp[:, :, D] = 1.0
    iota = np.arange(L)
    dead = (iota[None, :] >= vl[:, None]) & ~zmask[:, None]
    vp[dead] = 0.0
    vp = vp.astype(BF16NP)
    vp = np.ascontiguousarray(
        vp.reshape(n, KT, 128, D + 1).transpose(0, 2, 1, 3))
    return qkt, vp


def kernel(queries, keys, values, valid_lens):
    queries = np.asarray(queries, np.float32)
    keys = np.asarray(keys, np.float32)
    values = np.asarray(values, np.float32)
    vl = np.asarray(valid_lens).astype(np.int64)

    # Ragged load balancing: sort batches by active k-tile count descending,
    # deal across cores; slot s runs max-of-group tiles on every core.
    nact = np.where(vl == 0, KT, -(-vl // 128)).astype(np.int64)
    order = np.argsort(-nact, kind="stable")
    ns = tuple(int(nact[order[s * N_CORES]]) for s in range(BPC))

    qkt, vp = _prep_inputs(queries[order], keys[order], values[order],
                           vl[order])

    nc = get_program(ns)
    in_maps = []
    for c in range(N_CORES):
        idx = [s * N_CORES + c for s in range(BPC)]
        in_maps.append({
            "qkt": np.ascontiguousarray(qkt[idx]),
            "vp": np.ascontiguousarray(vp[idx]),
        })

    res = None
    for attempt in range(3):
        try:
            res = run_bass_kernel_spmd(nc, in_maps, list(range(N_CORES)))
            break
        except Exception:
            if attempt == 2:
                raise
            import time as _time
            _time.sleep(2.0)
            try:
                import jax
                jax.clear_caches()
            except Exception:
                pass

    out = np.empty((B, L, D), np.float32)
    for c in range(N_CORES):
        o = res.results[c]["o"]  # [BPC, 128, 2, 4, D+1]
        o = np.asarray(o, np.float32).reshape(BPC, 128, KT, D + 1)
        o = o.transpose(0, 2, 1, 3).reshape(BPC, L, D + 1)
        on = o[:, :, :D] / o[:, :, D:D + 1]
        for s in range(BPC):
            out[order[s * N_CORES + c]] = on[s]
    return out


# revision 27
# speedup vs baseline: 1.1797x; 1.1735x over previous
"""Masked dot-product attention (B=64, Lq=Lk=1024, d=64, fp32) on 8 TRN2 cores.

v2 strategy (per core: 8 batch slots, ragged k-tiles, sorted+dealt):
  - All inputs bf16. Host folds 1/sqrt(d) into Q. Masking is NOT in the
    score matmul: dead k rows (k >= valid_len) are zeroed in V (including
    the ones-column that produces softmax denominators), so whatever the
    exp stage emits for dead scores is multiplied by zero in the O matmul.
  - S^T[k,q] per k-tile via bf16 matmul (contraction d=64), PSUM f32.
  - exp is split across TWO engines to break the single-engine exp wall:
      ACT: exact exp (PSUM->SBUF bf16)
      DVE: Schraudolph fast-exp: i16 = rint(S*(2^7/ln2) + 127*2^7), whose
           bit pattern IS bf16(exp(S)) to ~3%; f32->i16 convert saturates
           (verified on HW) so dead scores (~-1e6) become 0x8000 = -0.0.
           The +3%-band bias cancels in the softmax division; using the
           uncorrected constant keeps exp(0)=1.0 exactly so valid_len==0
           batches (host zeroes Q) stay exactly uniform.
    Small batches (<=2 k-tiles) are ACT-only: Schraudolph error hurts most
    when few keys are live.
  - O^T[q,j] = sum_k P^T[k,q-chunk]^T V[k,j]: lhsT = P^T chunk [128,128],
    rhs = V-tile [128,65] (64 dims + ones column) -> out [128q, 65], only
    65 PE rows per matmul (vs 1024 streaming V^T P). PSUM accumulation
    groups clear has_written bank-wide on start, so the 8 q-chunks run as
    2 passes x 4 chunks, each chunk in its own PSUM bank ([128,4,512] f32
    tile, single buffer); pass1 re-reads the kept P tiles. O-work is a
    global FIFO drained between tiles so the PE stream never blocks on a
    PSUM buffer freed by later instructions.
  - copies PSUM->SBUF (engine chosen by load balance), output DMAs issued
    from GpSimd (SWDGE) keeping SP.SEQ/HWDGE for inputs only.
"""

import math
from collections import deque

import numpy as np
import ml_dtypes

import concourse.bass as bass
import concourse.mybir as mybir
import concourse.tile as tile
from concourse import bacc
from concourse.bass_utils import run_bass_kernel_spmd

N_CORES = 8
B = 64
L = 1024
D = 64
BPC = B // N_CORES
KT = L // 128

F32 = mybir.dt.float32
BF16 = mybir.dt.bfloat16
I16 = mybir.dt.int16
BF16NP = ml_dtypes.bfloat16

A16 = 128.0 / math.log(2.0)   # 184.6617
B16 = 127.0 * 128.0           # 16256.0

ACT_EXP_NS = 1038.0
DVE_EXP_NS = 1191.0
ACT_CP_NS = 293.0
DVE_CP_NS = 260.0

# structure knobs (swept via sweep.py)
O_UNIT = "bank"    # "bank" | "chunk"
O_LAG = 1
ZIPPER = True
HEAD_SPLIT = True
SP_BUFS = 3
PT_BUFS = 12
OP_BUFS = 2        # [128,4,128] one-bank tiles
EXEC_ORDER = [6, 0, 3, 1, 4, 2, 5, 7]

_prog_cache = {}


def _plan(ns):
    """Execution order, per-tile engine map, per-copy engine map."""
    # head: a small ACT-only batch; tail: the smallest batch (short drain).
    exec_order = EXEC_ORDER or ([BPC - 2] + list(range(BPC - 2)) + [BPC - 1])
    # weave the head: the second batch's first tiles go right after the
    # first batch's tile 0 (their DMA chunks land first), so neither exp
    # engine waits for the other batch's later input chunks.
    b0, b1 = exec_order[0], exec_order[1]
    if ZIPPER:
        z = [(b0, 0), (b1, 0), (b1, 1)]
        rest0 = [(b0, kt) for kt in range(1, ns[b0])]
        rest1 = [(b1, kt) for kt in range(2, ns[b1])]
        while rest0 or rest1:
            if rest0:
                z.append(rest0.pop(0))
            for _ in range(2):
                if rest1:
                    z.append(rest1.pop(0))
        tiles = z + [(b, kt) for b in exec_order[2:] for kt in range(ns[b])]
    else:
        tiles = [(b, kt) for b in exec_order for kt in range(ns[b])]
    busy = {"A": 0.0, "D": 0.0}
    eng = {}
    for (b, kt) in tiles:
        if ns[b] <= 2:
            e = "A"   # accuracy: few live keys -> exact exp
        elif b == b1 and kt < 2:
            e = "D"   # wake DVE early in the head
        elif busy["A"] + ACT_EXP_NS <= busy["D"] + DVE_EXP_NS:
            e = "A"
        else:
            e = "D"
        eng[(b, kt)] = e
        busy[e] += ACT_EXP_NS if e == "A" else DVE_EXP_NS
    cpeng = {}
    for b in exec_order:
        for p in range(4):
            if busy["A"] + ACT_CP_NS <= busy["D"] + DVE_CP_NS:
                cpeng[(b, p)] = "A"
                busy["A"] += ACT_CP_NS
            else:
                cpeng[(b, p)] = "D"
                busy["D"] += DVE_CP_NS
    return exec_order, tiles, eng, cpeng


def _build_program(ns):
    """ns: per-slot k-tile counts (tuple of BPC ints in 1..KT)."""
    nc = bacc.Bacc("TRN2", target_bir_lowering=False, debug=False,
                   num_devices=N_CORES)
    exec_order, tiles, eng, cpeng = _plan(ns)

    # qkt: [ktile0 (128) | qt (1024) | ktile1.. (896)] bf16, 64 partitions
    qkt_d = nc.dram_tensor("qkt", [BPC, D, 2 * L + 128], BF16,
                           kind="ExternalInput")
    vp_d = nc.dram_tensor("vp", [BPC, 128, KT, D + 1], BF16,
                          kind="ExternalInput")
    o_d = nc.dram_tensor("o", [BPC, 128, 2, 4, D + 1], F32,
                         kind="ExternalOutput")

    with tile.TileContext(nc) as tc:
        with (
            tc.tile_pool(name="qk", bufs=1) as qk_pool,
            tc.tile_pool(name="vpp", bufs=1) as vp_pool,
            tc.tile_pool(name="pt", bufs=PT_BUFS) as pt_pool,
            tc.tile_pool(name="osb", bufs=3) as osb_pool,
            tc.tile_pool(name="sp", bufs=SP_BUFS, space="PSUM") as sp_pool,
            tc.tile_pool(name="op", bufs=OP_BUFS, space="PSUM") as op_pool,
        ):
            qkt_s = {}
            vp_s = {}
            head2 = set(exec_order[:2])
            for b in exec_order:
                nkt = ns[b]
                q_t = qk_pool.tile([D, 2 * L + 128], BF16, tag=f"qkt{b}")
                v_t = vp_pool.tile([128, KT, D + 1], BF16, tag=f"vp{b}")
                qkt_s[b] = q_t
                vp_s[b] = v_t
            # head: first two batches get small first chunks (ktile0 + the
            # first 640 qt columns) so their first 512-wide S-matmuls (and
            # with them both exp engines) start as early as possible
            for b in exec_order[:2]:
                nc.sync.dma_start(qkt_s[b][:, :768], qkt_d[b][:, :768])
            for b in (exec_order[1], exec_order[0]):
                nkt = ns[b]
                end = 128 + L + (nkt - 1) * 128
                nc.sync.dma_start(qkt_s[b][:, 768:end], qkt_d[b][:, 768:end])
            for b in exec_order[:2]:
                nc.sync.dma_start(vp_s[b][:, :ns[b], :], vp_d[b][:, :ns[b], :])
            for b in exec_order[2:]:
                nkt = ns[b]
                end = 128 + L + (nkt - 1) * 128
                nc.sync.dma_start(qkt_s[b][:, :end], qkt_d[b][:, :end])
                nc.sync.dma_start(vp_s[b][:, :nkt, :], vp_d[b][:, :nkt, :])

            def ktm_sl(b, kt):
                if kt == 0:
                    return qkt_s[b][:, :128]
                o = 128 + L + (kt - 1) * 128
                return qkt_s[b][:, o:o + 128]

            def qt_sl(b):
                return qkt_s[b][:, 128:128 + L]

            state = {}  # b -> dict(op0/op1 tiles, osb, pt list)
            owork = deque()

            def emit_exp(b, kt, sp, pt, splits):
                e = eng[(b, kt)]
                if e == "A":
                    for (lo, hi) in splits:
                        nc.scalar.activation(
                            pt[:, lo:hi].bitcast(BF16), sp[:, lo:hi],
                            mybir.ActivationFunctionType.Exp)
                else:
                    for (lo, hi) in splits:
                        nc.vector.tensor_scalar(
                            pt[:, lo:hi], sp[:, lo:hi], A16, B16,
                            mybir.AluOpType.mult, mybir.AluOpType.add)

            last_b = exec_order[-1]

            def o_mm_pair(b, j):
                # O accumulation for q-chunks 2j, 2j+1. Chunks sharing a
                # PSUM bank run as SEQUENTIAL groups (all of chunk c's
                # matmuls before chunk c+1 starts), which is legal despite
                # the bank-wide has_written clear on each group start.
                def f():
                    st = state[b]
                    nkt = ns[b]
                    bk = j // 2
                    if j % 2 == 0:
                        st["op%d" % bk] = op_pool.tile(
                            [128, 4, 128], F32, tag="op",
                            name=f"op_{b}_{bk}")
                    op_t = st["op%d" % bk]
                    for c in (2 * j, 2 * j + 1):
                        for kt in range(nkt):
                            nc.tensor.matmul(
                                op_t[:, c % 4, :D + 1],
                                st["pt"][kt][:, c * 128:(c + 1) * 128]
                                .bitcast(BF16),
                                vp_s[b][:, kt, :],
                                start=(kt == 0), stop=(kt == nkt - 1))
                return f

            def o_copy_pair(b, j):
                # copy chunks 2j,2j+1 PSUM->SBUF; emitted a tile later than
                # their matmuls so the copy never blocks its engine's SEQ.
                def f():
                    st = state[b]
                    bk, half = divmod(j, 2)
                    e = cpeng[(b, j)]
                    dst = st["osb"][:, bk, 2 * half:2 * half + 2, :]
                    src = st["op%d" % bk][:, 2 * half:2 * half + 2, :D + 1]
                    if e == "A":
                        nc.scalar.copy(dst, src)
                    else:
                        nc.vector.tensor_copy(dst, src)
                    if half == 1:
                        dma_eng = nc.sync if b == last_b else nc.gpsimd
                        dma_eng.dma_start(o_d[b][:, bk, :, :],
                                          st["osb"][:, bk, :, :])
                return f

            for i, (b, kt) in enumerate(tiles):
                nkt = ns[b]
                if kt == 0:
                    state[b] = {
                        "pt": [],
                        "osb": osb_pool.tile([128, 2, 4, D + 1], F32,
                                             tag="osb", name=f"osb{b}"),
                    }
                sp = sp_pool.tile([128, L], F32, tag="sp")
                jobs = [(0, 512), (512, 1024)]
                if HEAD_SPLIT and i < 2:
                    esplits = jobs
                else:
                    esplits = [(0, 1024)]
                for (lo, hi) in jobs:
                    nc.tensor.matmul(sp[:, lo:hi], ktm_sl(b, kt),
                                     qt_sl(b)[:, lo:hi],
                                     start=True, stop=True)
                pt = pt_pool.tile([128, L], I16, tag="pt")
                state[b]["pt"].append(pt)
                emit_exp(b, kt, sp, pt, esplits)
                if kt == nkt - 1:
                    # O work for batch b becomes eligible a few tiles
                    # later: by then its last exp has completed (sp
                    # recycling bounds exp lag), so the in-order PE queue
                    # never stalls on it. Each copy trails its matmul pair
                    # by a tile so it dispatches when already satisfiable.
                    for j in range(4):
                        owork.append((i + O_LAG + j, o_mm_pair(b, j)))
                        owork.append((i + O_LAG + j + 1, o_copy_pair(b, j)))
                while owork and owork[0][0] <= i:
                    owork.popleft()[1]()
            while owork:
                owork.popleft()[1]()

    nc.compile()
    return nc


def get_program(ns):
    ns = tuple(ns)
    if ns not in _prog_cache:
        _prog_cache[ns] = _build_program(ns)
    return _prog_cache[ns]


def _prep_inputs(q, k, v, vl):
    """q,k,v: [n, L, D] f32; vl: [n] int. Returns (qkt, vp) bf16 arrays."""
    n = q.shape[0]
    qkt = np.zeros((n, D, 2 * L + 128), BF16NP)
    qt = (q.transpose(0, 2, 1) * np.float32(1.0 / np.sqrt(D))).astype(BF16NP)
    zmask = vl == 0
    if zmask.any():
        qt[zmask] = 0
    ktm = k.transpose(0, 2, 1).astype(BF16NP)
    qkt[:, :, :128] = ktm[:, :, :128]
    qkt[:, :, 128:128 + L] = qt
    qkt[:, :, 128 + L:2 * L] = ktm[:, :, 128:]
    vp = np.empty((n, L, D + 1), np.float32)
    vp[:, :, :D] = v
    vp[:, :, D] = 1.0
    iota = np.arange(L)
    dead = (iota[None, :] >= vl[:, None]) & ~zmask[:, None]
    vp[dead] = 0.0
    vp = vp.astype(BF16NP)
    vp = np.ascontiguousarray(
        vp.reshape(n, KT, 128, D + 1).transpose(0, 2, 1, 3))
    return qkt, vp


def kernel(queries, keys, values, valid_lens):
    queries = np.asarray(queries, np.float32)
    keys = np.asarray(keys, np.float32)
    values = np.asarray(values, np.float32)
    vl = np.asarray(valid_lens).astype(np.int64)

    # Ragged load balancing: sort batches by active k-tile count descending,
    # deal across cores; slot s runs max-of-group tiles on every core.
    nact = np.where(vl == 0, KT, -(-vl // 128)).astype(np.int64)
    order = np.argsort(-nact, kind="stable")
    ns = tuple(int(nact[order[s * N_CORES]]) for s in range(BPC))

    qkt, vp = _prep_inputs(queries[order], keys[order], values[order],
                           vl[order])

    nc = get_program(ns)
    in_maps = []
    for c in range(N_CORES):
        idx = [s * N_CORES + c for s in range(BPC)]
        in_maps.append({
            "qkt": np.ascontiguousarray(qkt[idx]),
            "vp": np.ascontiguousarray(vp[idx]),
        })

    res = None
    for attempt in range(3):
        try:
            res = run_bass_kernel_spmd(nc, in_maps, list(range(N_CORES)))
            break
        except Exception:
            if attempt == 2:
                raise
            import time as _time
            _time.sleep(2.0)
            try:
                import jax
                jax.clear_caches()
            except Exception:
                pass

    out = np.empty((B, L, D), np.float32)
    for c in range(N_CORES):
        o = res.results[c]["o"]  # [BPC, 128, 2, 4, D+1]
        o = np.asarray(o, np.float32).reshape(BPC, 128, KT, D + 1)
        o = o.transpose(0, 2, 1, 3).reshape(BPC, L, D + 1)
        on = o[:, :, :D] / o[:, :, D:D + 1]
        for s in range(BPC):
            out[order[s * N_CORES + c]] = on[s]
    return out
